# revision 28
# baseline (speedup 1.0000x reference)
"""Embedding lookup (weight[input_ids]) on 8 Trainium2 NeuronCores.

Strategy: data-parallel over tokens. The 4x2048=8192 token ids are split
into 8 shards of 1024 tokens; every core holds the full [32000, 128] f32
table in HBM and uses the SWDGE dma_gather instruction to pull its 1024
rows (512 B each) directly from HBM into SBUF, then stores the gathered
block to its output shard with fully-contiguous DMAs.

Token->SBUF placement is chosen on the host so the SBUF->HBM store is
contiguous: gather position j handles token t = (j%128)*8 + j//128, which
lands token t's row at SBUF [partition t//8, block t%8].  Partition p then
holds tokens p*8..p*8+7 back to back, so the store AP is a plain
[128, 1024]f32 -> flat DRAM copy and the output shard comes out in natural
token order.

The gather is split into chunks so the SBUF->HBM store of chunk i overlaps
the gather of chunk i+1 (stores ride HWDGE on SP/ACT, gathers ride SWDGE
on Pool).  The gpsimd ucode library load is issued first so it overlaps
the ids load.
"""

import numpy as np

VOCAB = 32000
EMBED = 128
N_CORES = 8
B, S = 4, 2048
N = B * S                 # 8192 tokens total
NPC = N // N_CORES        # 1024 tokens per core
BLK = NPC // 128          # 8 blocks of 128 gather positions
IDXW = NPC // 16          # 64 idx columns in the wrapped idx layout

DEFAULT_CHUNKS = (640, 384)

_NC_CACHE = {}


def build_nc(chunk_sizes=DEFAULT_CHUNKS, split_store=False, no_gpsimd_drain=False,
             ids_drain_handoff=True, no_store_wait=True,
             strip_const_memsets=True, warmup_gather=False):
    """Build the per-core Bass program (identical on all 8 cores)."""
    from contextlib import ExitStack

    import concourse.bacc as bacc
    import concourse.mybir as mybir
    from concourse import library_config

    chunk_sizes = tuple(chunk_sizes)
    assert sum(chunk_sizes) == NPC
    assert all(c % 128 == 0 for c in chunk_sizes)
    chunks = len(chunk_sizes)
    starts = [sum(chunk_sizes[:i]) for i in range(chunks)]

    nc = bacc.Bacc("TRN2", target_bir_lowering=False, num_devices=N_CORES)

    ids_d = nc.dram_tensor("ids", [128, IDXW], mybir.dt.int16, kind="ExternalInput")
    w_d = nc.dram_tensor(
        "weight", [VOCAB, EMBED], mybir.dt.float32, kind="ExternalInput"
    )
    out_d = nc.dram_tensor(
        "out", [NPC, EMBED], mybir.dt.float32, kind="ExternalOutput"
    )

    with ExitStack() as stack:
        block = stack.enter_context(nc.Block(no_gpsimd_drain=no_gpsimd_drain))
        ids_sem = stack.enter_context(nc.semaphore("ids_sem"))
        ids_dma_sem = stack.enter_context(nc.semaphore("ids_dma_sem"))
        st_sem = stack.enter_context(nc.semaphore("st_sem"))
        gath_sems = [
            stack.enter_context(nc.semaphore(f"gath_sem{c}")) for c in range(chunks)
        ]
        idx_t = stack.enter_context(
            nc.sbuf_tensor("idx_t", [128, IDXW], mybir.dt.int16)
        )
        gath_t = stack.enter_context(
            nc.sbuf_tensor("gath_t", [128, NPC], mybir.dt.float32)
        )
        if warmup_gather:
            wu_sem = stack.enter_context(nc.semaphore("wu_sem"))
            wu_dma_sem = stack.enter_context(nc.semaphore("wu_dma_sem"))
            wu_idx = stack.enter_context(
                nc.sbuf_tensor("wu_idx", [128, 1], mybir.dt.int16)
            )
            wu_out = stack.enter_context(
                nc.sbuf_tensor("wu_out", [128, EMBED], mybir.dt.float32)
            )

        out_v = out_d.ap().rearrange("(p x) e -> p (x e)", p=128)  # [128, NPC]

        @block.gpsimd
        def _(g):
            g.load_library(library_config.mlp)
            # hoist the num_idxs registers so the ids wait attaches to the
            # first gather, not a register move
            regs = {}
            for ch in sorted(set(chunk_sizes)):
                regs[ch] = g.to_reg(ch)
            if warmup_gather:
                # run the gather ucode path once (row 0, 16 idxs) while the
                # ids DMA is in flight -- warms the Q7 icache off the
                # critical path
                g.memset(wu_idx[:], 0).then_inc(wu_sem, 1)
                g.wait_ge(wu_sem, 1)
                g.dma_gather(
                    wu_out[:].rearrange("p (b e) -> p b e", e=EMBED),
                    w_d.ap(),
                    wu_idx[:],
                    16,
                    g.to_reg(16),
                    EMBED,
                ).then_inc(wu_dma_sem, 16)
            g.wait_ge(ids_sem, 16)
            for c, (st, ch) in enumerate(zip(starts, chunk_sizes)):
                g.dma_gather(
                    gath_t[:, st : st + ch].rearrange("p (b e) -> p b e", e=EMBED),
                    w_d.ap(),
                    idx_t[:, st // 16 : (st + ch) // 16],
                    ch,           # num_idxs
                    regs[ch],     # num_idxs_reg (all indices valid)
                    EMBED,        # elem_size (one table row)
                ).then_inc(gath_sems[c], 16)

        @block.sync
        def _(sp):
            if ids_drain_handoff:
                # drain waits for the HWDGE FIFO (data landed), then a cheap
                # engine sem-inc signals Pool -- skips the 900ns DMA sem prop
                sp.dma_start(idx_t[:], ids_d.ap()).then_inc(ids_dma_sem, 16)
                sp.drain().then_inc(ids_sem, 16)
            else:
                sp.dma_start(idx_t[:], ids_d.ap()).then_inc(ids_sem, 16)
            for c, (st, ch) in enumerate(zip(starts, chunk_sizes)):
                if split_store and c % 2 == 1:
                    continue
                sp.wait_ge(gath_sems[c], 16)
                sp.dma_start(
                    out_v[:, st : st + ch], gath_t[:, st : st + ch]
                ).then_inc(st_sem, 16)
            if not no_store_wait:
                sp.wait_ge(st_sem, 16 * chunks)

        if split_store:

            @block.scalar
            def _(act):
                for c, (st, ch) in enumerate(zip(starts, chunk_sizes)):
                    if c % 2 == 0:
                        continue
                    act.wait_ge(gath_sems[c], 16)
                    act.dma_start(
                        out_v[:, st : st + ch], gath_t[:, st : st + ch]
                    ).then_inc(st_sem, 16)

    if strip_const_memsets:
        # The framework preamble memsets four const-* SBUF tiles this kernel
        # never reads; dropping them shortens the Pool preamble before the
        # entry barrier.
        import concourse.mybir as mybir

        blk = nc.m.functions[0].blocks[0]
        keep = [
            i
            for i in blk.instructions
            if not (
                isinstance(i, mybir.InstMemset)
                and i.outs
                and str(getattr(i.outs[0], "memref", "")).startswith("const-")
            )
        ]
        blk.instructions = keep

    nc.compile()
    return nc


def _get_nc():
    if "nc" not in _NC_CACHE:
        _NC_CACHE["nc"] = build_nc()
    return _NC_CACHE["nc"]


def prep_ids(ids_flat):
    """Per-core wrapped int16 idx arrays for dma_gather.

    Gather position j of core c looks up token t(j) = (j%128)*8 + j//128 of
    the core's shard.  dma_gather reads idx j from SBUF partition j%16,
    column j//16, replicated across the 8 gpsimd cores (16 partitions each).
    """
    per_core = []
    for c in range(N_CORES):
        shard = ids_flat[c * NPC : (c + 1) * NPC]
        pos = shard.reshape(128, BLK).T.reshape(-1)      # pos[j] = shard[t(j)]
        wrapped = pos.reshape(IDXW, 16).T                # [16, 64]
        full = np.tile(wrapped, (8, 1)).astype(np.int16)  # [128, 64]
        per_core.append(np.ascontiguousarray(full))
    return per_core


def run_spmd(inputs, trace=False, nc=None):
    """Returns (output [4,2048,128] f32, BassKernelResults)."""
    from concourse.bass_utils import run_bass_kernel_spmd

    ids = np.asarray(inputs["input_ids"]).reshape(-1).astype(np.int64)
    w = np.ascontiguousarray(np.asarray(inputs["weight"], dtype=np.float32))
    assert ids.shape == (N,) and w.shape == (VOCAB, EMBED)

    in_maps = [{"ids": ids_c, "weight": w} for ids_c in prep_ids(ids)]
    res = run_bass_kernel_spmd(
        nc if nc is not None else _get_nc(),
        in_maps,
        core_ids=list(range(N_CORES)),
        trace=trace,
    )
    shards = [r["out"] for r in res.results]
    out = np.concatenate(shards, axis=0).reshape(B, S, EMBED)
    return np.ascontiguousarray(out.astype(np.float32)), res


def kernel(**inputs):
    out, _ = run_spmd(inputs, trace=False)
    return out
